# revision 59
# baseline (speedup 1.0000x reference)
"""Trainium2 Bass kernel for nn_BNESNN_46213848105973.

Reference computation (B = 1048576, W0 = 6*I(12), W1 = 0, W2 = 6*I(24)):
    s0  = (x1 @ W0 > 0.5)          elementwise spike: (6*x1 > 0.5)   [B, 12]
    s1  = (s0 @ W1 + x2 @ W2 > .5) elementwise spike: (6*x2 > 0.5)   [B, 24]
    dw1 = s0^T @ s1                                                  [12, 24]
    dw2 = x2^T @ s1                                                  [24, 24]
Returns (dw1, dw2, s0, s1).

Sharding: pure data parallel over 8 NeuronCores (batch dim). Each core
computes its spike shards plus partial dw sums (PSUM-accumulated matmuls
contracted over its 131072 rows); the 8 tiny [24,36] partials are summed
on the host during unshard.
"""

import numpy as np

B = 1048576
NCORES = 8
BS = B // NCORES      # 131072 rows per core
P = 128               # SBUF partitions
NPP = BS // P         # 1024 batch rows per partition
# Group schedule (chunks of 128 rows per group): large groups amortize
# per-DMA overhead; the shrinking tail keeps the end-of-kernel chain
# (last in-DMA -> spike -> out-DMA/matmul -> dw store) short.
# (group_size, prefetched): prefetched groups get dedicated SBUF slots and
# are emitted first, so their spikes finish mid-kernel and their out-DMAs
# close the kernel with zero spike-wait.
SCHEDULE = [(176, False)] * 5 + [(96, False), (48, True)]
assert sum(g for g, _ in SCHEDULE) == NPP
_IO_BUFS = 3
D1 = 12
D2 = 24

_cache = {}


def _build_nc(scale1: float, thresh1: float, scale2: float, thresh2: float):
    import concourse.mybir as mybir
    import concourse.tile as tile
    from concourse import bacc

    f32 = mybir.dt.float32
    bf16 = mybir.dt.bfloat16
    u8 = mybir.dt.uint8

    nc = bacc.Bacc(
        "TRN2",
        target_bir_lowering=False,
        debug=False,
        num_devices=NCORES,
    )

    x1s = nc.dram_tensor("x1s", [BS, D1], f32, kind="ExternalInput").ap()
    x2s = nc.dram_tensor("x2s", [BS, D2], f32, kind="ExternalInput").ap()
    # Spikes are exactly 0/1: write them as uint8 (4x fewer HBM bytes) and
    # upcast to f32 on the host during unshard — bit-exact either way.
    s0s = nc.dram_tensor("s0s", [BS, D1], u8, kind="ExternalOutput").ap()
    s1s = nc.dram_tensor("s1s", [BS, D2], u8, kind="ExternalOutput").ap()
    dwt = nc.dram_tensor("dwt", [24, 36], f32, kind="ExternalOutput").ap()

    # Partition-major views: partition p holds rows [p*NPP, (p+1)*NPP), each
    # partition's span is contiguous in DRAM.
    x1v = x1s.rearrange("(p n) d -> p (n d)", p=P)
    x2v = x2s.rearrange("(p n) d -> p (n d)", p=P)
    s0v = s0s.rearrange("(p n) d -> p (n d)", p=P)
    s1v = s1s.rearrange("(p n) d -> p (n d)", p=P)

    # Batch offsets follow SCHEDULE order; emission order puts the
    # prefetched tail groups first.
    offsets = []
    start_chunk = 0
    for G, pre in SCHEDULE:
        offsets.append((G, pre, start_chunk))
        start_chunk += G
    emit_order = [e for e in offsets if e[1]] + [e for e in offsets if not e[1]]
    total_chunks = sum(g for g, _ in SCHEDULE)

    with tile.TileContext(nc) as tc:
        with (
            tc.tile_pool(name="io", bufs=_IO_BUFS) as io_pool,
            tc.tile_pool(name="pre", bufs=1) as pre_pool,
            tc.tile_pool(name="psum", bufs=1, space="PSUM") as psum_pool,
            tc.tile_pool(name="dw", bufs=1) as dw_pool,
        ):
            # One PSUM accumulator [24, 36] in a single bank: cols 0:12
            # accumulate s1^T@s0 (dw1^T), cols 12:36 accumulate s1^T@x2
            # (dw2^T). One accumulation group — has_written bits make the
            # first write to each element an overwrite — so the tail needs
            # only one PSUM->SBUF copy.
            p_dw = psum_pool.tile([24, D1 + D2], f32, tag="p_dw")

            emitted = 0
            deferred_outs = []
            for G, pre, start_chunk in emit_order:
                c1 = slice(start_chunk * D1, (start_chunk + G) * D1)
                c2 = slice(start_chunk * D2, (start_chunk + G) * D2)
                pool = pre_pool if pre else io_pool
                sfx = f"_p{start_chunk}" if pre else ""

                x1t = pool.tile([P, G * D1], f32, tag="x1t" + sfx)
                nc.sync.dma_start(x1t[:], x1v[:, c1])
                x2t = pool.tile([P, G * D2], f32, tag="x2t" + sfx)
                nc.sync.dma_start(x2t[:], x2v[:, c2])

                # uint8 spike tiles feed the DMA-out of s0/s1 (values 0/1).
                s0t = pool.tile([P, G * D1], u8, tag="s0t" + sfx)
                nc.vector.tensor_scalar(
                    s0t[:], x1t[:], scale1, thresh1,
                    op0=mybir.AluOpType.mult, op1=mybir.AluOpType.is_gt,
                )
                s1t = pool.tile([P, G * D2], u8, tag="s1t" + sfx)
                nc.vector.tensor_scalar(
                    s1t[:], x2t[:], scale2, thresh2,
                    op0=mybir.AluOpType.mult, op1=mybir.AluOpType.is_gt,
                )
                # bf16 PE feeds: s0b/x2b convert on the otherwise-idle ACT
                # engine (0/1 exact in any dtype); s1b spikes on DVE. bf16
                # matmuls stream 1 cycle/row; spikes are exact in bf16 so dw1
                # is exact and dw2 only sees x2's bf16 rounding (~1e-6 after
                # the 1M-row fp32 PSUM accumulation).
                # The last-emitted group's feeds+matmuls are split into halves
                # (separate tiles) so its stop-matmul — which gates the dwt
                # store at the kernel tail — fires ~1us earlier.
                is_tail_group = (G, pre, start_chunk) == emit_order[-1]
                n_halves = 2 if is_tail_group else 1
                h_off = 0
                for h in range(n_halves):
                    Gh = G // n_halves if h < n_halves - 1 else G - h_off
                    hs1 = slice(h_off * D1, (h_off + Gh) * D1)
                    hs2 = slice(h_off * D2, (h_off + Gh) * D2)
                    hsx = f"{sfx}_h{h}" if n_halves > 1 else sfx
                    s0b = pool.tile([P, Gh * D1], bf16, tag="s0b" + hsx)
                    nc.scalar.copy(s0b[:], s0t[:, hs1])
                    s1b = pool.tile([P, Gh * D2], bf16, tag="s1b" + hsx)
                    nc.vector.tensor_scalar(
                        s1b[:], x2t[:, hs2], scale2, thresh2,
                        op0=mybir.AluOpType.mult, op1=mybir.AluOpType.is_gt,
                    )
                    x2b = pool.tile([P, Gh * D2], bf16, tag="x2b" + hsx)
                    nc.scalar.copy(x2b[:], x2t[:, hs2])

                    for n in range(Gh):
                        first = emitted + h_off + n == 0
                        last = emitted + h_off + n == total_chunks - 1
                        s1n = s1b[:, n * D2:(n + 1) * D2]
                        nc.tensor.matmul(
                            p_dw[:, 0:D1], s1n, s0b[:, n * D1:(n + 1) * D1],
                            start=first, stop=False,
                        )
                        nc.tensor.matmul(
                            p_dw[:, D1:D1 + D2], s1n, x2b[:, n * D2:(n + 1) * D2],
                            start=False, stop=last,
                        )
                    h_off += Gh
                emitted += G

                if pre:
                    # Prefetched groups' outs are emitted at the very end on
                    # the SP ring: ready early, they fill the final main
                    # group's spike-wait gap without queueing behind ACT.
                    deferred_outs.append((c1, c2, s0t, s1t))
                else:
                    # Outputs go out on the Activation HWDGE ring so stalled
                    # out-DMAs (waiting on spikes) never block input issue
                    # on SP.
                    nc.scalar.dma_start(s0v[:, c1], s0t[:])
                    nc.scalar.dma_start(s1v[:, c2], s1t[:])

            for c1, c2, s0t, s1t in deferred_outs:
                nc.sync.dma_start(s0v[:, c1], s0t[:])
                nc.sync.dma_start(s1v[:, c2], s1t[:])

            # dwt leaves on the SP ring: ACT is still draining the last s1s
            # transfer at this point while SP is idle.
            dwsb = dw_pool.tile([24, 36], f32, tag="dwsb")
            nc.vector.tensor_copy(dwsb[:], p_dw[:])
            nc.sync.dma_start(dwt[:], dwsb[:])

    nc.compile()
    return nc


def kernel(x1, x2, W0=None, W1=None, W2=None, **_ignored):
    from concourse.bass_utils import run_bass_kernel_spmd

    x1 = np.ascontiguousarray(np.asarray(x1, dtype=np.float32))
    x2 = np.ascontiguousarray(np.asarray(x2, dtype=np.float32))
    assert x1.shape == (B, D1) and x2.shape == (B, D2)

    # W0/W2 are diagonal scalings and W1 is zero by construction; bake the
    # scales as immediates (threshold 0.5 from the IFNode default). If the
    # structure ever differs, fall back to a host computation rather than
    # returning a silently wrong answer.
    scale1 = float(np.asarray(W0)[0, 0]) if W0 is not None else 6.0
    scale2 = float(np.asarray(W2)[0, 0]) if W2 is not None else 6.0
    if W0 is not None and W1 is not None and W2 is not None:
        W0 = np.asarray(W0, dtype=np.float32)
        W1 = np.asarray(W1, dtype=np.float32)
        W2 = np.asarray(W2, dtype=np.float32)
        if not (
            np.array_equal(W0, np.diag(np.full(D1, scale1, np.float32)))
            and not W1.any()
            and np.array_equal(W2, np.diag(np.full(D2, scale2, np.float32)))
        ):
            s0 = (x1 @ W0 > 0.5).astype(np.float32)
            s1 = (s0 @ W1 + x2 @ W2 > 0.5).astype(np.float32)
            dw1 = (s0.T.astype(np.float64) @ s1.astype(np.float64)).astype(np.float32)
            dw2 = (x2.T.astype(np.float64) @ s1.astype(np.float64)).astype(np.float32)
            return dw1, dw2, s0, s1
    key = (scale1, scale2)
    if key not in _cache:
        _cache[key] = _build_nc(scale1, 0.5, scale2, 0.5)
    nc = _cache[key]

    in_maps = [
        {
            "x1s": x1[c * BS:(c + 1) * BS],
            "x2s": x2[c * BS:(c + 1) * BS],
        }
        for c in range(NCORES)
    ]
    res = run_bass_kernel_spmd(nc, in_maps, core_ids=list(range(NCORES)))
    outs = res.results

    s0 = np.concatenate([outs[c]["s0s"] for c in range(NCORES)], axis=0).astype(
        np.float32
    )
    s1 = np.concatenate([outs[c]["s1s"] for c in range(NCORES)], axis=0).astype(
        np.float32
    )
    dwt = np.sum(
        np.stack([outs[c]["dwt"] for c in range(NCORES)]).astype(np.float64), axis=0
    )
    dw1 = np.ascontiguousarray(dwt[:, 0:D1].T).astype(np.float32)
    dw2 = np.ascontiguousarray(dwt[:, D1:D1 + D2].T).astype(np.float32)
    return dw1, dw2, s0, s1
